# revision 1
# baseline (speedup 1.0000x reference)
"""Bezier Gaussian-splat raster kernel for 8 Trainium2 NeuronCores.

Reference computation (RES=1024, STEPS=256, SIGMA=0.01):
    curve = bezier(control_points)            # (2, 256)
    Ex[a,s] = exp(-(g[a]-x[s])^2 / (2 sigma^2))   # (1024, 256)
    Ey[b,s] = exp(-(g[b]-y[s])^2 / (2 sigma^2))
    OUT     = (Ey @ Ex^T) / 256               # (1024, 1024)  == raster.T

Sharding: 4 row-blocks x 2 col-blocks = 8 cores. Core i handles output rows
[256*(i//2), +256) and cols [512*(i%2), +512).

Design notes (per core):
  - One tiny input DMA: block-shifted control points broadcast to 128
    partitions, [128, 16] f32. Grids are iota-generated block-LOCAL indices;
    since the Bezier basis is a partition of unity, shifting the control
    points by the block offset shifts the curve identically, so no other
    per-core data is needed.
  - Bezier basis is computed on device from a [128, 2] iota; curve points are
    elementwise basis*control-point products summed on DVE.
  - Exponent args stay fp32: arg = (2c x'/RES)*j - Square(sqrt(c)/RES * j)
    (+ per-point bias -c x'^2 inside the ACT exp). exp outputs are fp16.
  - The 1/STEPS scale rides the y-side exp biases (-ln S).
  - 256-contraction fp16 matmuls (2 s-chunks x 2 m-chunks, N=512) write the
    final output into PSUM; ACT and DVE evacuate one m-chunk each and the two
    stores go out on the two HWDGE rings in parallel.
"""

import math

import numpy as np

import concourse.bacc as bacc
import concourse.bass as bass
import concourse.mybir as mybir
import concourse.tile as tile
from concourse.bass_utils import run_bass_kernel_spmd

RES = 1024
STEPS = 256
SIGMA = 0.01
INV2S2 = 1.0 / (2.0 * SIGMA * SIGMA)  # 5000.0
SQC = math.sqrt(INV2S2)
LN_S = math.log(STEPS)

R_BLK = 4
C_BLK = 2
MROWS = RES // R_BLK  # 256
NCOLS = RES // C_BLK  # 512
N_CORES = 8

F32 = mybir.dt.float32
F16 = mybir.dt.float16
I16 = mybir.dt.int16

G_DTYPE = F16

_CACHE: dict = {}


def _build_nc() -> bass.Bass:
    # Skip the ~3µs all-engine EVSEM barrier Bass.__init__ emits after its
    # const-AP memsets; our first const-AP use is µs later.
    _orig_barrier = bass.Bass.all_engine_barrier
    bass.Bass.all_engine_barrier = lambda self, **kw: None
    try:
        nc = bacc.Bacc(
            "TRN2",
            target_bir_lowering=False,
            debug=False,
            enable_asserts=False,
            enable_partition_id=False,
        )
    finally:
        bass.Bass.all_engine_barrier = _orig_barrier

    # cols 0:12 block-shifted control points k-duplicated
    # (cp[j,d] - block_offset[d] at col k*6+j*2+d), rest pad.
    cpk = nc.dram_tensor("cpk", [128, 16], F32, kind="ExternalInput").ap()
    out = nc.dram_tensor("out", [MROWS, NCOLS], F32, kind="ExternalOutput").ap()

    MULT = mybir.AluOpType.mult
    ADD = mybir.AluOpType.add
    SUB = mybir.AluOpType.subtract
    EXP = mybir.ActivationFunctionType.Exp
    SQUARE = mybir.ActivationFunctionType.Square

    with tile.TileContext(nc) as tc:
        with (
            tc.tile_pool(name="const", bufs=1) as cpool,
            tc.tile_pool(name="work", bufs=1) as wpool,
            tc.tile_pool(name="ps", bufs=1, space="PSUM") as ppool,
        ):
            # --- the one input DMA, on the ACT HWDGE ring, issued first ----
            cpk_sb = cpool.tile([128, 16], F32)
            nc.scalar.dma_start(cpk_sb[:], cpk)

            # --- early ACT exp-table load trigger --------------------------
            scratch = cpool.tile([128, 2], F32)
            nc.gpsimd.memset(scratch[:], 0.0)
            nc.scalar.activation(scratch[:, 1:2], scratch[:, 0:1], EXP)

            # --- iota grids (int16 indices, block-local) -------------------
            sPk = cpool.tile([128, 2], I16)
            nc.gpsimd.iota(sPk[:], [[128, 2]], base=0, channel_multiplier=1)
            gxi = cpool.tile([128, NCOLS], I16)
            nc.gpsimd.iota(gxi[:], [[1, NCOLS]], base=0, channel_multiplier=0)
            gyi = cpool.tile([128, MROWS], I16)
            nc.gpsimd.iota(gyi[:], [[1, MROWS]], base=0, channel_multiplier=0)

            # --- +c*(j/RES)^2 via ACT Square -------------------------------
            cg2x = wpool.tile([128, NCOLS], F32, tag="cg2x")
            nc.scalar.activation(cg2x[:], gxi[:], SQUARE, scale=SQC / RES)
            cg2y = wpool.tile([128, MROWS], F32, tag="cg2y")
            nc.scalar.activation(cg2y[:], gyi[:], SQUARE, scale=SQC / RES)

            # --- Bezier basis on DVE (s = 128k + p) ------------------------
            # B3[p, 2j+k] = basis_j(s); u = s/255 (linspace), v = s/256
            u = wpool.tile([128, 2], F32)
            nc.vector.tensor_scalar(u[:], sPk[:], 1.0 / 255.0, None, MULT)
            v = wpool.tile([128, 2], F32)
            nc.vector.tensor_scalar(v[:], sPk[:], 1.0 / 256.0, None, MULT)
            su = wpool.tile([128, 2], F32)
            nc.vector.tensor_scalar(su[:], u[:], -1.0, 1.0, MULT, ADD)
            sv = wpool.tile([128, 2], F32)
            nc.vector.tensor_scalar(sv[:], v[:], -1.0, 1.0, MULT, ADD)
            B3 = wpool.tile([128, 6], F32)
            nc.vector.tensor_tensor(B3[:, 0:2], su[:], sv[:], MULT)  # c0
            nc.vector.tensor_tensor(B3[:, 4:6], u[:], v[:], MULT)  # c2
            c02 = wpool.tile([128, 2], F32)
            nc.vector.tensor_tensor(c02[:], B3[:, 0:2], B3[:, 4:6], ADD)
            nc.vector.tensor_scalar(B3[:, 2:4], c02[:], -1.0, 1.0, MULT, ADD)  # c1

            # --- curve points (shifted by block offsets) -------------------
            # prods[p, k*6+j*2+d] = basis_j(s_k) * cp[j, d]
            b3a = B3[:, 0:6]
            in0 = bass.AP(
                b3a.tensor, b3a.offset, [list(b3a.ap[0]), [1, 2], [2, 3], [0, 2]]
            )
            prods = wpool.tile([128, 12], F32)
            nc.vector.tensor_tensor(prods[:], in0, cpk_sb[:, 0:12], MULT)
            # The basis is a partition of unity (c0+c1+c2 = 1), so the host
            # pre-subtracts each core's block offset from the control points;
            # the summed products are directly the block-local curve points.
            # One reduce over the re-striped (k, d, j) view sums the 3 basis
            # products per coordinate: xy4[p, 2k+d] = block-local curve.
            pa = prods[:, 0:12]
            pv2 = bass.AP(
                pa.tensor, pa.offset, [list(pa.ap[0]), [6, 2], [1, 2], [2, 3]]
            )
            # The host also pre-scales the control points by 2c/RES, so the
            # reduce directly yields B'[p, 2k+d] = (2c/RES) * curve' — the stt
            # scalar coefficients — with no further per-point scaling op.
            xy4 = wpool.tile([128, 4], F32)
            nc.vector.reduce_sum(xy4[:], pv2, axis=mybir.AxisListType.X)

            # --- exp biases: C2 = -c xy'^2 = -(RES^2/4c) B'^2 (-lnS on y) ---
            bc = wpool.tile([128, 4], F32)
            nc.vector.scalar_tensor_tensor(
                bc[:], xy4[:], -(RES * RES) / (4.0 * INV2S2), xy4[:], MULT, MULT
            )
            nc.vector.tensor_scalar(bc[:, 1:2], bc[:, 1:2], LN_S, None, SUB)
            nc.vector.tensor_scalar(bc[:, 3:4], bc[:, 3:4], LN_S, None, SUB)

            # --- exponent args + exp ---------------------------------------
            gxe = []
            gye = []
            for k in range(2):
                argx = ppool.tile([128, NCOLS], F32, tag=f"argx{k}", name=f"argx{k}")
                nc.vector.scalar_tensor_tensor(
                    argx[:], gxi[:], xy4[:, 2 * k : 2 * k + 1], cg2x[:], MULT, SUB
                )
                ex = wpool.tile([128, NCOLS], G_DTYPE, tag=f"gxe{k}")
                nc.scalar.activation(
                    ex[:], argx[:], EXP, bias=bc[:, 2 * k : 2 * k + 1]
                )
                gxe.append(ex)

                argy = ppool.tile([128, MROWS], F32, tag=f"argy{k}", name=f"argy{k}")
                nc.vector.scalar_tensor_tensor(
                    argy[:], gyi[:], xy4[:, 2 * k + 1 : 2 * k + 2], cg2y[:],
                    MULT, SUB
                )
                ey = wpool.tile([128, MROWS], G_DTYPE, tag=f"gye{k}")
                nc.scalar.activation(
                    ey[:], argy[:], EXP, bias=bc[:, 2 * k + 1 : 2 * k + 2]
                )
                gye.append(ey)

            # --- matmul: OUT[m, n] = sum_s Ey[s, m] * Ex[s, n] -------------
            pouts = [
                ppool.tile([128, NCOLS], F32, tag=f"pout{m}", name=f"pout{m}")
                for m in range(2)
            ]
            for k in range(2):
                for m in (1, 0):
                    nc.tensor.matmul(
                        pouts[m][:],
                        gye[k][:, 128 * m : 128 * (m + 1)],
                        gxe[k][:],
                        start=(k == 0),
                        stop=(k == 1),
                        skip_group_check=True,
                    )

            # --- evacuate + store (parallel engines + HWDGE rings) ---------
            out1 = wpool.tile([128, NCOLS], F32, tag="out1")
            nc.vector.tensor_copy(out1[:], pouts[1][:])
            nc.sync.dma_start(out[128:256, :], out1[:])
            out0 = wpool.tile([128, NCOLS], F32, tag="out0")
            nc.scalar.copy(out0[:], pouts[0][:])
            nc.scalar.dma_start(out[0:128, :], out0[:])

    nc.compile()
    return nc


def _get_cached():
    if "nc" not in _CACHE:
        _CACHE["nc"] = _build_nc()
    return _CACHE["nc"]


def kernel(control_points: np.ndarray, _trace: bool = False):
    nc = _get_cached()
    cp = np.asarray(control_points, dtype=np.float32)
    assert cp.shape == (3, 2)

    in_maps = []
    for i in range(N_CORES):
        r, c = i // C_BLK, i % C_BLK
        off = np.array(
            [(c * NCOLS) / RES, (r * MROWS) / RES], dtype=np.float32
        )
        flat = (
            (cp - off[None, :]) * np.float32(2.0 * INV2S2 / RES)
        ).reshape(-1).astype(np.float32)
        row = np.zeros((1, 16), dtype=np.float32)
        row[0, 0:6] = flat
        row[0, 6:12] = flat
        in_maps.append(
            {"cpk": np.ascontiguousarray(np.broadcast_to(row, (128, 16)))}
        )

    res = run_bass_kernel_spmd(
        nc, in_maps, core_ids=list(range(N_CORES)), trace=_trace
    )
    _CACHE["last_results"] = res

    full = np.empty((RES, RES), dtype=np.float32)
    for i in range(N_CORES):
        r, c = i // C_BLK, i % C_BLK
        full[r * MROWS : (r + 1) * MROWS, c * NCOLS : (c + 1) * NCOLS] = res.results[
            i
        ]["out"]
    return full

